# revision 112
# baseline (speedup 1.0000x reference)
"""AttentionBlock kernel for Trainium2, sharded over 8 NeuronCores.

Problem (hardcoded shapes): x [b=4, c=1024, t=1024] fp32
  GroupNorm(32 groups) -> 1x1 conv qkv (3072x1024) -> 16-head attention
  (head dim 64, scale ch**-0.25 on both q and k) -> 1x1 proj -> residual.

Sharding: core = (batch, head-half).  Core 2*b+g handles batch b and heads
8g..8g+7.  Host sums the four per-core partial outputs per batch
(h = proj DR-pair kt0+kt2, h3 = DR-pair kt1+kt3, both bf16) and adds the
residual x and proj bias itself (free on host; removed the
identity-residual matmuls and let h ship at half the bytes).

Numerics: the projections (qkv, v^T, attention@v, output proj) run in
fp8e4m3 with DoubleRow perf mode -- one instruction contracts two K-planes
at 2x the bf16 matmul rate, PROVIDED the two rhs planes are contiguous
512-column blocks (xn8/asb8/pr8 are laid out n2-major for this; a strided
plane pair measured 2x slower).  Scores stay bf16 (K=64 wastes half the PE
rows either way; fp8-DR at 32 partitions measured 2x SLOWER).  Probs are
fp8 with a constant bias shift exp(s-3): overflow-safe (max score ~6.7,
TRN fp8e4m3 max 240) and exactly cancelled by the ones-column denominator.
Some exp steps per head (DVE_ST: 2 for heads 0-3, 3 for heads 4-7 -- 20
of 64, the measured optimum) run on the DVE as a bit-trick (bits =
round(s*8log2e + B) as u8 ~ fp8 of exp(s-3)) to offload the saturated
ACT queue; measured err impact ~nil.  Weights are pre-scaled by
8 and activations (a) by 16 before the fp8 cast to dodge the subnormal
band; the scales fold into existing evacuation scalars.  End-to-end l2
rel err ~5.2e-3 (gate 2e-2).

Measured lessons (traces): the BSP preamble is a fixed ~7us; per-queue
DMA runs ~100GB/s so x rides 2-tile 4KB-descriptor chunks on all three
queues; HAM starts the PE at 1.2GHz and only sustained-busy windows
un-throttle it, so warm-up matmuls gated on the first x chunk bridge
into the stats burst; a head-pair-interleaved scores pipeline (disjoint
PE row groups for LDW pull-ahead) was tried and is FASTER on paper but
its fine-grained stream never gives HAM a contiguous busy window -- the
whole mid phase ran at 1.2GHz and lost 40us.  Keep PE work in dense
bursts.

Engine discipline learned from traces:
  - ACT queue carries ONLY the 64 softmax exps mid-kernel (its in-order
    queue head-of-line-blocks on anything with unmet deps); the epilogue
    divide runs af-evac(DVE) -> ones-matmul denominator broadcast (PE,
    K=1 -- gpsimd partition_broadcast lives in a different Pool ucode
    library than tensor_mul and each per-head swap stalled ~6us) ->
    recip(DVE) -> mul(Pool, its only op class, one resident library).
  - DMA-issue instructions on the scalar queue execute on the ACT
    sequencer and are ring-throttled, so the scalar queue gets only the
    odd x tiles; remaining weights ride sync after the even x tiles or
    the gpsimd SWDGE queue gated behind the last x tile.
  - GroupNorm stats finalize per 4-tile half (gsel is block-diagonal) so
    xn8 and the m0 q/k chains start before the second half's x arrives.

Schedule: flat (head, st) software pipeline with scores leading av by one
step; v^T tiles fill head 0, q/k chains heads 1-3, proj wave A (DR pair
kt0+kt2, needs head 5, shipped as bf16 h) heads 6-7; wave B (DR pair
kt1+kt3, bf16 h3) drains after head 7 with per-half pipelining.  Pairing
the proj waves as (0+2)/(1+3) instead of (0+1)+kt2+kt3 halves their
matmul cycles, removes 8 evacuations from ACT exactly where it was the
serial engine, and drops a whole 2MB output (h2).  Each head epilogue's
pden/recip/asb8 part is deferred one step so its PE ones-matmuls never
head-of-line-block the next head's scores (that stall made HAM
re-throttle the clock at every head boundary).  PSUM: scores
double-buffer (4 banks) + av accumulator/denominator-broadcast slot (2) +
background chain slot (2).
"""

import numpy as np
import ml_dtypes

import concourse.bass as bass
import concourse.tile as tile
from concourse import bacc, mybir
from concourse.bass_utils import run_bass_kernel_spmd

F32 = mybir.dt.float32
BF16 = mybir.dt.bfloat16
F8 = mybir.dt.float8e4
U8 = mybir.dt.uint8
AF = mybir.ActivationFunctionType
ALU = mybir.AluOpType
AX = mybir.AxisListType
DR = mybir.MatmulPerfMode.DoubleRow

B, C, T = 4, 1024, 1024
GROUPS = 32
N_HEADS = 16
CH = C // N_HEADS            # 64
EPS = 1e-5
NCORES = 8
HPC = 8                      # heads per core
CPC = HPC * CH               # 512
CT = C // 128                # 8 c-tiles
TT = T // 128                # 8 t-tiles
GSIZE = C // GROUPS
GN_N = GSIZE * T

SHIFT = 3.0                  # exp(s - SHIFT); cancels in the denominator
WSC = 8.0                    # weight pre-scale before fp8 cast
ASC = 16.0                   # activation (a) pre-scale before fp8 cast
SCALE = float(CH) ** -0.25
L2E = float(np.log2(np.e))
EXP_A = 8.0 * L2E            # DVE bit-trick: bits = round(s*EXP_A + EXP_B)
EXP_B = 8.0 * (7.0 - SHIFT * L2E - 0.0435)

# steps whose exp runs on the DVE (bit trick) instead of ACT, per head
# steps whose exp runs on the DVE (bit trick) instead of ACT, per head.
# Heads 4-7 give the DVE more: that's where waveA/waveBk2 evacs pile onto
# ACT and the pipeline was measurably ACT-serial (balances ACT/DVE at
# ~86us each).
DVE_ST = {h: ((2, 5) if h < 4 else (2, 4, 6)) for h in range(HPC)}

_CACHE = {}


def _build_program():
    nc = bacc.Bacc("TRN2", target_bir_lowering=False, debug=False, num_devices=NCORES)

    names = [
        ("xb", [128, CT, T], BF16),
        ("gsel", [128, 128], BF16),
        ("wq8", [128, CT, CPC], F8),
        ("wk8", [128, CT, CPC], F8),
        ("wv8", [128, CT, CPC], F8),
        ("bqk", [128, 8], F32),
        ("bvb", [128, HPC, CH], BF16),
        ("pt8", [128, 4, C], F8),
    ]
    aps = {}
    for n, shp, dt in names:
        aps[n] = nc.dram_tensor(n, shp, dt, kind="ExternalInput").ap()
    aps["h"] = nc.dram_tensor("h", [CT, 128, T], BF16, kind="ExternalOutput").ap()
    aps["h3"] = nc.dram_tensor("h3", [CT, 128, T], BF16, kind="ExternalOutput").ap()

    with tile.TileContext(nc) as tc:
        _body(tc, aps)
    nc.compile()
    return nc


def _body(tc, aps):
    nc = tc.nc
    with (
        tc.tile_pool(name="wpool", bufs=1) as wpool,
        tc.tile_pool(name="xpool", bufs=1) as xpool,
        tc.tile_pool(name="stats", bufs=1) as stats,
        tc.tile_pool(name="qk", bufs=1) as qk,
        tc.tile_pool(name="probs", bufs=4) as probsp,
        tc.tile_pool(name="bc", bufs=2) as bcp,
        tc.tile_pool(name="hp", bufs=4) as hp,
        tc.tile_pool(name="pp", bufs=1, space="PSUM") as pp,
    ):
        # ---- loads: x tiles alternate the two DMA queues; weights follow
        # (gsel first on scalar -- the stats matmul needs it early) ----
        xb = xpool.tile([128, CT, T], BF16)
        gsel_t = wpool.tile([128, 128], BF16)
        wq8_t = wpool.tile([128, CT, CPC], F8)
        wk8_t = wpool.tile([128, CT, CPC], F8)
        bqk_t = wpool.tile([128, 8], F32)
        wv8_t = wpool.tile([128, CT, CPC], F8)
        bvb_t = wpool.tile([128, HPC, CH], BF16)
        pt8_t = wpool.tile([128, 4, C], F8)
        # DMA queues: gsel first on sync (tiny; the stats matmul gates on
        # it), then x tiles round-robin over FOUR queues (sync, scalar,
        # gpsimd-SWDGE, vector) so the 2MB of x lands in ~1/4 the time;
        # stats half 0 needs tiles 0-3, which land first (one per queue).
        # Weights follow x on each queue, spread so no queue carries more
        # than ~1MB before its last-needed-early tensor.
        # x rides in 2-tile chunks (4KB contiguous per partition -- ~2x the
        # per-queue descriptor throughput of 2KB tiles), one chunk per
        # queue so stats half 0 (tiles 0-3) lands first everywhere.
        # the scalar queue gets ONLY the x23 issue: a second DMA-issue
        # there ring-throttles the ACT sequencer behind x23's descriptors
        # (~4us) and delays the whole stats chain.  wk8/bqk ride sync
        # behind wq8 -- they land ~18us, which is off the critical path
        # (the m0 k-chain is gated by xn8 j3 anyway).
        nc.sync.dma_start(out=gsel_t, in_=aps["gsel"])
        nc.sync.dma_start(out=xb[:, 0:2, :], in_=aps["xb"][:, 0:2, :])
        nc.sync.dma_start(out=wq8_t, in_=aps["wq8"])
        nc.sync.dma_start(out=wk8_t, in_=aps["wk8"])
        nc.sync.dma_start(out=bqk_t, in_=aps["bqk"])
        nc.scalar.dma_start(out=xb[:, 2:4, :], in_=aps["xb"][:, 2:4, :])
        nc.gpsimd.dma_start(out=xb[:, 4:6, :], in_=aps["xb"][:, 4:6, :])
        nc.gpsimd.dma_start(out=xb[:, 6:8, :], in_=aps["xb"][:, 6:8, :])
        nc.gpsimd.dma_start(out=wv8_t, in_=aps["wv8"])
        nc.gpsimd.dma_start(out=bvb_t, in_=aps["bvb"])
        nc.gpsimd.dma_start(out=pt8_t, in_=aps["pt8"])

        # ---- PE warm-up: ~8 junk matmuls gated on a mid-arriving x tile.
        # HAM starts the PE throttled to 1.2GHz and only un-throttles after
        # ~3.4us of sustained busy; these bridge from x-arrival into the
        # stats matmul so the real matmuls run at 2.4GHz.  (Warming too
        # early -- e.g. on gsel at ~8us -- just re-throttles again before
        # the stats burst.)  Rides the bg psum slot; its WAR sem vs the
        # stats matmul is fine. ----
        wps = pp.tile([128, T], F32, tag="bg", name="ps_warm")
        for _ in range(8):
            nc.tensor.matmul(wps[:, 0:512], lhsT=xb[:, 0, 0:128],
                             rhs=xb[:, 0, 0:512], start=True, stop=True)

        epst = wpool.tile([128, 1], F32)
        nc.vector.memset(epst, EPS)
        # dummy activations: pull the ACT table loads (~1.3us each) into
        # the idle DMA window instead of the critical stats/exp path.
        # Square last so its set is resident when the real squares start.
        tldw = wpool.tile([128, 1], F32)
        nc.scalar.activation(out=tldw, in_=epst, func=AF.Exp)
        nc.scalar.activation(out=tldw, in_=epst, func=AF.Identity)
        nc.scalar.activation(out=tldw, in_=epst, func=AF.Square)

        nsh = wpool.tile([128, 1], F32)
        nc.vector.memset(nsh, -SHIFT)
        ascol = wpool.tile([128, 1], F32)
        nc.vector.memset(ascol[0:64], 1.0)
        nc.vector.memset(ascol[64:128], ASC)
        onesr = wpool.tile([1, 128], BF16)
        nc.vector.memset(onesr, 1.0)



        # ---- GroupNorm stats (affine folded into qkv on host).  gsel is
        # block-diagonal (groups live within a c-tile), so stats finalize
        # per 4-tile half: xn8 + the m0 q/k chain segments start ~5us
        # earlier.  sstf layout: [sum0-3 | sq0-3 | sum4-7 | sq4-7]. ----
        # per-tile sum (DVE) + sumsq (ACT) in separate tiles (a shared one
        # serialized the engines through the dependency tracker)
        sstf_s = stats.tile([128, CT], F32)
        sstf_q = stats.tile([128, CT], F32)
        for i in range(CT):
            nc.vector.reduce_sum(out=sstf_s[:, i:i + 1], in_=xb[:, i, :],
                                 axis=AX.X)
            sq = stats.tile([128, T], F32, tag="sq", bufs=2, name="sq")
            nc.scalar.activation(out=sq, in_=xb[:, i, :], func=AF.Square,
                                 accum_out=sstf_q[:, i:i + 1])
        sst = stats.tile([128, 2 * CT], BF16)
        pstat = pp.tile([128, T], F32, tag="bg", name="pstat")
        mean = stats.tile([128, CT], F32)
        var = stats.tile([128, CT], F32)
        std = stats.tile([128, CT], F32)
        rstd = stats.tile([128, CT], F32)
        msq = stats.tile([128, CT], F32)
        nmr = stats.tile([128, CT], F32)

        def emit_stats_half(hf):
            c0 = 4 * hf
            sls = slice(c0, c0 + 4)
            nc.vector.tensor_copy(out=sst[:, 8 * hf:8 * hf + 4],
                                  in_=sstf_s[:, sls])
            nc.vector.tensor_copy(out=sst[:, 8 * hf + 4:8 * hf + 8],
                                  in_=sstf_q[:, sls])
            nc.tensor.matmul(pstat[:, 8 * hf:8 * hf + 8], lhsT=gsel_t,
                             rhs=sst[:, 8 * hf:8 * hf + 8],
                             start=True, stop=True)
            nc.vector.tensor_scalar_mul(mean[:, sls],
                                        pstat[:, 8 * hf:8 * hf + 4],
                                        1.0 / GN_N)
            nc.vector.tensor_mul(msq[:, sls], mean[:, sls], mean[:, sls])
            nc.vector.scalar_tensor_tensor(out=var[:, sls],
                                           in0=pstat[:, 8 * hf + 4:8 * hf + 8],
                                           scalar=1.0 / GN_N, in1=msq[:, sls],
                                           op0=ALU.mult, op1=ALU.subtract)
            nc.scalar.activation(out=std[:, sls], in_=var[:, sls],
                                 func=AF.Sqrt, bias=epst)
            nc.vector.reciprocal_approx_fast(out=rstd[:, sls],
                                             in_=std[:, sls])
            nc.vector.scalar_tensor_tensor(out=nmr[:, sls], in0=mean[:, sls],
                                           scalar=-1.0, in1=rstd[:, sls],
                                           op0=ALU.mult, op1=ALU.mult)

        # ---- persistent activation tiles ----
        xn8 = xpool.tile([128, 4, 2, 2, 512], F8)
        qsb = qk.tile([128, 4, T], BF16)
        ksb = qk.tile([128, 4, T], BF16)
        asb8 = qk.tile([128, 2, 2, 2, 512], F8)
        # v^T, DR-packed: [s(128), st-pair, plane, head, col].  Col 0 = ones:
        # the softmax denominator lands on psum partition 0, where gpsimd
        # partition_broadcast reads.  Cols 1-63 zero pad, 64-127 = v: the num
        # rows sit at base partition 64 (a 64-partition Pool access must be
        # 64-aligned).
        # memsets ride Pool (memset ucode is resident from init; DVE was
        # paying 3.4us for the zero-fill it can't afford mid-front)
        vt8 = qk.tile([128, 4, 2, HPC, 128], F8)
        nc.gpsimd.memset(vt8[:, :, :, :, 0:1], 1.0)
        nc.gpsimd.memset(vt8[:, :, :, :, 1:64], 0.0)

        def psum_tile(tag, bufs=1):
            return pp.tile([128, T], F32, tag=tag, bufs=bufs, name=f"ps_{tag}")

        # ---- xn8 (fp8) + m0 q/k chains, pipelined per stats-half ----
        qm0 = psum_tile("sc", 2)
        km0 = psum_tile("sc", 2)
        for j in range(4):
            if j % 2 == 0:
                emit_stats_half(j // 2)
            i0, i1 = 2 * j, 2 * j + 1
            # xn8 split 4 DVE / 4 ACT: balances the two front EW chains
            # (DVE: 8 reduces + 4 xn8 ~= ACT: 8 squares + 4 xn8)
            if j < 2:
                nc.vector.tensor_scalar(out=xn8[:, j, :, 0, :],
                                        in0=xb[:, i0, :],
                                        scalar1=mean[:, i0:i0 + 1],
                                        scalar2=rstd[:, i0:i0 + 1],
                                        op0=ALU.subtract, op1=ALU.mult)
                nc.vector.tensor_scalar(out=xn8[:, j, :, 1, :],
                                        in0=xb[:, i1, :],
                                        scalar1=mean[:, i1:i1 + 1],
                                        scalar2=rstd[:, i1:i1 + 1],
                                        op0=ALU.subtract, op1=ALU.mult)
            else:
                nc.scalar.activation(out=xn8[:, j, :, 0, :], in_=xb[:, i0, :],
                                     func=AF.Identity, bias=nmr[:, i0:i0 + 1],
                                     scale=rstd[:, i0:i0 + 1])
                nc.scalar.activation(out=xn8[:, j, :, 1, :], in_=xb[:, i1, :],
                                     func=AF.Identity, bias=nmr[:, i1:i1 + 1],
                                     scale=rstd[:, i1:i1 + 1])
            for ps, wt in ((qm0, wq8_t), (km0, wk8_t)):
                for n2 in range(2):
                    nc.tensor.matmul(
                        ps[:, n2 * 512:(n2 + 1) * 512],
                        lhsT=wt[:, i0:i0 + 2, 0:128],
                        rhs=xn8[:, j, n2, :, :],
                        perf_mode=DR, start=(j == 0), stop=(j == 3))
        nc.vector.tensor_scalar(out=qsb[:, 0, :], in0=qm0,
                                scalar1=SCALE / WSC, scalar2=bqk_t[:, 0:1],
                                op0=ALU.mult, op1=ALU.add)
        nc.scalar.activation(out=ksb[:, 0, :], in_=km0, func=AF.Identity,
                             bias=bqk_t[:, 4:5], scale=SCALE / WSC)

        def emit_vt(tt):
            ps = psum_tile("bg")
            n2, tb = tt // 4, tt % 4
            for j in range(4):
                nc.tensor.matmul(
                    ps[:, 0:CPC],
                    lhsT=xn8[:, j, n2, :, tb * 128:(tb + 1) * 128],
                    rhs=wv8_t[:, 2 * j:2 * j + 2, :],
                    perf_mode=DR, start=(j == 0), stop=(j == 3))
            nc.vector.scalar_tensor_tensor(
                out=vt8[:, tt // 2, tt % 2, :, 64:128],
                in0=ps[:, 0:CPC].rearrange("p (h c) -> p h c", h=HPC),
                scalar=1.0 / WSC, in1=bvb_t, op0=ALU.mult, op1=ALU.add)

        qk_chain = {}

        def emit_qk_seg(m, which, j):
            wt, bcol, dst = ((wq8_t, m, qsb), (wk8_t, 4 + m, ksb))[which]
            key = (m, which)
            if key not in qk_chain:
                qk_chain[key] = psum_tile("bg")
            ps = qk_chain[key]
            for n2 in range(2):
                nc.tensor.matmul(
                    ps[:, n2 * 512:(n2 + 1) * 512],
                    lhsT=wt[:, 2 * j:2 * j + 2, m * 128:(m + 1) * 128],
                    rhs=xn8[:, j, n2, :, :],
                    perf_mode=DR, start=(j == 0), stop=(j == 3))
            if j == 3:
                nc.vector.tensor_scalar(out=dst[:, m, :], in0=ps,
                                        scalar1=SCALE / WSC,
                                        scalar2=bqk_t[:, bcol:bcol + 1],
                                        op0=ALU.mult, op1=ALU.add)
                del qk_chain[key]

        def emit_waveA(ot):
            # DR pair (kt0, kt2) = heads 0,1,4,5: runs during heads 6-7
            # (needs head 5's epilogue).  Residual + proj bias are added
            # on the HOST (it sums the partials anyway), so h ships bf16.
            ps = psum_tile("bg")
            for n2 in range(2):
                sl = slice(n2 * 512, (n2 + 1) * 512)
                nc.tensor.matmul(
                    ps[:, sl],
                    lhsT=pt8_t[:, 0:2, ot * 128:(ot + 1) * 128],
                    rhs=asb8[:, 0, n2, :, :],
                    perf_mode=DR, start=True, stop=True)
            ho = hp.tile([128, T], BF16, tag="ho")
            nc.scalar.activation(out=ho, in_=ps, func=AF.Identity,
                                 scale=1.0 / (WSC * ASC))
            nc.sync.dma_start(out=aps["h"][ot], in_=ho)

        wb_state = {}

        def emit_waveB_half(ot, n2):
            srcs = [("sc", 2), ("sc", 2), ("bg", 1), ("pa", 1)]
            if n2 == 0:
                tag, bufs = srcs[ot % 4]
                wb_state[ot] = (psum_tile(tag, bufs),
                                hp.tile([128, T], BF16, tag="hs", name="hs"))
            ps, hs = wb_state[ot]
            sl = slice(n2 * 512, (n2 + 1) * 512)
            nc.tensor.matmul(ps[:, sl],
                             lhsT=pt8_t[:, 2:4, ot * 128:(ot + 1) * 128],
                             rhs=asb8[:, 1, n2, :, :],
                             perf_mode=DR, start=True, stop=True)
            if n2 == 0:
                nc.vector.tensor_scalar_mul(hs[:, sl], ps[:, sl],
                                            1.0 / (WSC * ASC))
            else:
                nc.scalar.activation(out=hs[:, sl], in_=ps[:, sl],
                                     func=AF.Identity, scale=1.0 / (WSC * ASC))
                del wb_state[ot]
            # ship each half as soon as it exists: n2=0 halves on sync;
            # n2=1 halves split gpsimd/scalar (all n2=1 emits come after
            # the last asb8 mul in Pool program order, so no head-of-line
            # risk; ACT's tail is light enough to afford 4 issues)
            if n2 == 0:
                eng = nc.sync
            else:
                eng = nc.gpsimd if ot % 2 == 0 else nc.scalar
            eng.dma_start(out=aps["h3"][ot][:, sl], in_=hs[:, sl])

        # Background schedule: (head, st) -> closures run between that
        # step's scores and the lagged av, where PE would otherwise wait.
        sched = {}

        def at(h, st, fn):
            sched.setdefault((h, st), []).append(fn)

        for j in range(6):
            at(0, j, lambda tt=j + 2: emit_vt(tt))
        for m in (1, 2, 3):
            for j in range(4):
                at(m, j, lambda m=m, j=j: emit_qk_seg(m, 0, j))
                at(m, 4 + j, lambda m=m, j=j: emit_qk_seg(m, 1, j))
        # wave A' (kt0+kt2 DR) needs head 5's epilogue -> heads 6-7 slots
        for ot in range(CT):
            at(6 + ot // 4, 1 + 2 * (ot % 4), lambda ot=ot: emit_waveA(ot))

        # ---- lead-in v^T tiles ----
        for tt in range(2):
            emit_vt(tt)

        # ---- attention: flat (head, st) pipeline, scores lead av by 1 ----
        steps = [(h, st) for h in range(HPC) for st in range(TT)]
        pa_tiles = {}
        pr_tiles = {}

        def emit_scores(k):
            h, st = steps[k]
            m, po = h // 2, CH * (h % 2)
            if st % 2 == 0:
                pr_tiles[k // 2] = probsp.tile([128, 2, 2, 512], F8, tag="pr",
                                               name="prt")
            ps = psum_tile("sc", 2)
            for n2 in range(2):
                nc.tensor.matmul(
                    ps[:, n2 * 512:(n2 + 1) * 512],
                    lhsT=ksb[po:po + CH, m, st * 128:(st + 1) * 128],
                    rhs=qsb[po:po + CH, m, n2 * 512:(n2 + 1) * 512],
                    start=True, stop=True)
            pr = pr_tiles[k // 2]
            psr = ps.rearrange("p (a n) -> p a n", a=2)
            if st in DVE_ST[h]:
                nc.vector.tensor_scalar(out=pr[:, :, st % 2, :].bitcast(U8),
                                        in0=psr, scalar1=EXP_A, scalar2=EXP_B,
                                        op0=ALU.mult, op1=ALU.add)
            else:
                nc.scalar.activation(out=pr[:, :, st % 2, :], in_=psr,
                                     func=AF.Exp, bias=nsh)

        def emit_av(k):
            h, st = steps[k]
            m, po = h // 2, CH * (h % 2)
            if st % 2 == 0:
                return
            j = st // 2
            pr = pr_tiles.pop(k // 2)
            if j == 0:
                # heads 4-5 accumulate in the bg slot (free there: no
                # chains or waves), so their pa allocation never WARs the
                # previous head's pden/rc2 chain -- that WAR stalled the
                # av stream ~4us at the boundaries into heads 4-6
                pa_tiles[h] = pp.tile([128, T], F32,
                                      tag=("bg" if h == 4 else "pa"),
                                      bufs=1, name="pat")
            pa = pa_tiles[h]
            for n2 in range(2):
                nc.tensor.matmul(
                    pa[0:128, n2 * 512:(n2 + 1) * 512],
                    lhsT=vt8[:, j, :, h, :],
                    rhs=pr[:, n2, :, :],
                    perf_mode=DR, start=(j == 0), stop=(j == 3))
            if st != TT - 1:
                return
            # head epilogue part 1: pa row 0 = denominator, rows 64-127 =
            # num*ASC after the ascol-scaled evac (frees the single pa
            # slot).  Part 2 (pden broadcast / recip / asb8) is DEFERRED
            # one step: its PE ones-matmuls wait on this evac, and emitted
            # inline they head-of-line-blocked the next head's scores for
            # >1us at EVERY head boundary -- exactly the per-head PE idle
            # that made HAM re-throttle the clock to 1.2GHz mid-kernel.
            af = bcp.tile([128, T], BF16, tag="af")
            rc2 = bcp.tile([128, T], F32, tag="rc2")
            nc.vector.tensor_scalar_mul(af[:, 0:512], pa[0:128, 0:512], ascol)
            if h < HPC - 1:
                nc.vector.tensor_scalar_mul(af[:, 512:T], pa[0:128, 512:T],
                                            ascol)
            else:
                nc.scalar.activation(out=af[:, 512:T], in_=pa[0:128, 512:T],
                                     func=AF.Identity, scale=ascol)
            epi_pend.append((h, af, rc2))

        epi_pend = []

        def epi_part2(h, af, rc2):
            m, po = h // 2, CH * (h % 2)
            # pden reuses the tag its head's pa lived in (just freed by
            # the af evac), never the next head's slot
            pden = pp.tile([128, T], F32,
                           tag=("bg" if h == 4 else "pa"),
                           bufs=1, name="pden")
            for nb in range(2):
                sl = slice(nb * 512, (nb + 1) * 512)
                nc.tensor.matmul(pden[:, sl], lhsT=onesr, rhs=af[0:1, sl],
                                 start=True, stop=True)
                nc.vector.reciprocal_approx_fast(out=rc2[:, sl],
                                                 in_=pden[:, sl])
                # asb8 index: [po, wave m%2, nb, plane m//2, :] -- each
                # proj wave is a DR pair (kt0+kt2) / (kt1+kt3) and DR
                # wants its two planes adjacent (strided pairs are 2x
                # slower), so heads interleave by m parity
                nc.gpsimd.tensor_mul(
                    out=asb8[po:po + CH, m % 2, nb, m // 2, :],
                    in0=af[64:128, sl], in1=rc2[64:128, sl])
                if h == HPC - 1 and nb == 0:
                    # kick wave B's first halves as soon as its n2=0
                    # planes exist
                    for ot in range(2):
                        emit_waveB_half(ot, 0)

        LEAD = 1
        for k in range(64 + LEAD):
            if k < 64:
                emit_scores(k)
            if epi_pend:
                # before the sched closures: waveA/waveBk2 in this slot
                # read the asb8 this writes
                epi_part2(*epi_pend.pop(0))
            if k < 64:
                for fn in sched.get(steps[k], ()):
                    fn()
            if k >= LEAD:
                emit_av(k - LEAD)
        while epi_pend:
            epi_part2(*epi_pend.pop(0))

        # ---- wave B tail: DR pair kt1+kt3 (kt0+kt2 shipped during heads
        # 6-7); psum slots rotate so ot chains overlap; evacs DVE/ACT ----
        for ot in range(2, CT):
            emit_waveB_half(ot, 0)
        for ot in range(CT):
            emit_waveB_half(ot, 1)


def _pack_inputs(x, gn_weight, gn_bias, qkv_w, qkv_b, proj_w, proj_b):
    """Build the 8 per-core input dicts (host-side packing only)."""
    bf = ml_dtypes.bfloat16
    f8 = ml_dtypes.float8_e4m3
    gsel = np.kron(np.eye(4, dtype=np.float32),
                   np.ones((GSIZE, GSIZE), dtype=np.float32)).astype(bf)
    # Fold GroupNorm affine into the qkv conv (exact):
    qkv_b = (qkv_b.astype(np.float64) +
             qkv_w.astype(np.float64) @ gn_bias.astype(np.float64)
             ).astype(np.float32)
    qkv_w = (qkv_w * gn_weight[None, :]).astype(np.float32)

    in_maps = []
    for core in range(NCORES):
        b_idx, g = core // 2, core % 2
        hh = np.arange(CPC) // CH + HPC * g
        cc = np.arange(CPC) % CH
        qrows = 192 * hh + cc
        krows = qrows + CH
        vrows = qrows + 2 * CH

        def packT(rows):
            w = (qkv_w[rows, :] * WSC).T.astype(f8)          # [C, CPC]
            return np.ascontiguousarray(
                w.reshape(CT, 128, CPC).transpose(1, 0, 2))  # [128, CT, CPC]

        bqv = np.concatenate(
            [(qkv_b[qrows] * SCALE).reshape(4, 128).T,
             (qkv_b[krows] * SCALE).reshape(4, 128).T], axis=1)
        bvv = np.ascontiguousarray(np.broadcast_to(
            qkv_b[vrows].reshape(1, HPC, CH), (128, HPC, CH))).astype(bf)

        ptm = (proj_w[:, g * CPC:(g + 1) * CPC].T * WSC).astype(f8)
        # kt plane order (0, 2, 1, 3): the two proj waves are DR pairs
        # (kt0+kt2) and (kt1+kt3)
        ptm = np.ascontiguousarray(
            ptm.reshape(4, 128, C)[[0, 2, 1, 3]].transpose(1, 0, 2))

        xin = np.ascontiguousarray(
            x[b_idx].reshape(CT, 128, T).transpose(1, 0, 2).astype(bf))

        in_maps.append({
            "xb": xin,
            "gsel": gsel,
            "wq8": packT(qrows),
            "wk8": packT(krows),
            "wv8": packT(vrows),
            "bqk": np.ascontiguousarray(bqv.astype(np.float32)),
            "bvb": bvv,
            "pt8": ptm,
        })
    return in_maps


def kernel(x, gn_weight, gn_bias, qkv_w, qkv_b, proj_w, proj_b, **run_kwargs):
    x = np.asarray(x, dtype=np.float32)
    gn_weight = np.asarray(gn_weight, dtype=np.float32)
    gn_bias = np.asarray(gn_bias, dtype=np.float32)
    qkv_w = np.asarray(qkv_w, dtype=np.float32)
    qkv_b = np.asarray(qkv_b, dtype=np.float32)
    proj_w = np.asarray(proj_w, dtype=np.float32)
    proj_b = np.asarray(proj_b, dtype=np.float32)

    if "nc" not in _CACHE:
        _CACHE["nc"] = _build_program()
    nc = _CACHE["nc"]

    in_maps = _pack_inputs(x, gn_weight, gn_bias, qkv_w, qkv_b, proj_w, proj_b)
    res = run_bass_kernel_spmd(nc, in_maps, core_ids=list(range(NCORES)),
                               **run_kwargs)
    out = np.empty((B, C, T), dtype=np.float32)
    for b_idx in range(B):
        r0, r1 = res.results[2 * b_idx], res.results[2 * b_idx + 1]
        acc = x[b_idx] + proj_b[:, None]
        for r in (r0, r1):
            for nm in ("h", "h3"):
                acc = acc + np.asarray(r[nm]).reshape(C, T).astype(np.float32)
        out[b_idx] = acc
    if run_kwargs:
        return out, res
    return out



# revision 113
# speedup vs baseline: 1.0500x; 1.0500x over previous
"""AttentionBlock kernel for Trainium2, sharded over 8 NeuronCores.

Problem (hardcoded shapes): x [b=4, c=1024, t=1024] fp32
  GroupNorm(32 groups) -> 1x1 conv qkv (3072x1024) -> 16-head attention
  (head dim 64, scale ch**-0.25 on both q and k) -> 1x1 proj -> residual.

Sharding: core = (batch, head-half).  Core 2*b+g handles batch b and heads
8g..8g+7.  Host sums the four per-core partial outputs per batch
(h = proj DR-pair kt0+kt2, h3 = DR-pair kt1+kt3, both bf16) and adds the
residual x and proj bias itself (free on host; removed the
identity-residual matmuls and let h ship at half the bytes).

Numerics: the projections (qkv, v^T, attention@v, output proj) run in
fp8e4m3 with DoubleRow perf mode -- one instruction contracts two K-planes
at 2x the bf16 matmul rate, PROVIDED the two rhs planes are contiguous
512-column blocks (xn8/asb8/pr8 are laid out n2-major for this; a strided
plane pair measured 2x slower).  Scores stay bf16 (K=64 wastes half the PE
rows either way; fp8-DR at 32 partitions measured 2x SLOWER).  Probs are
fp8 with a constant bias shift exp(s-3): overflow-safe (max score ~6.7,
TRN fp8e4m3 max 240) and exactly cancelled by the ones-column denominator.
Some exp steps per head (DVE_ST: 2 for heads 0-3, 3 for heads 4-7 -- 20
of 64, the measured optimum) run on the DVE as a bit-trick (bits =
round(s*8log2e + B) as u8 ~ fp8 of exp(s-3)) to offload the saturated
ACT queue; measured err impact ~nil.  Weights are pre-scaled by
8 and activations (a) by 16 before the fp8 cast to dodge the subnormal
band; the scales fold into existing evacuation scalars.  End-to-end l2
rel err ~5.2e-3 (gate 2e-2).

Measured lessons (traces): the BSP preamble is a fixed ~7us; per-queue
DMA runs ~100GB/s so x rides 2-tile 4KB-descriptor chunks on all three
queues; HAM starts the PE at 1.2GHz and only sustained-busy windows
un-throttle it, so warm-up matmuls gated on the first x chunk bridge
into the stats burst; a head-pair-interleaved scores pipeline (disjoint
PE row groups for LDW pull-ahead) was tried and is FASTER on paper but
its fine-grained stream never gives HAM a contiguous busy window -- the
whole mid phase ran at 1.2GHz and lost 40us.  Keep PE work in dense
bursts.

Engine discipline learned from traces:
  - ACT queue carries ONLY the 64 softmax exps mid-kernel (its in-order
    queue head-of-line-blocks on anything with unmet deps); the epilogue
    divide runs af-evac(DVE) -> ones-matmul denominator broadcast (PE,
    K=1 -- gpsimd partition_broadcast lives in a different Pool ucode
    library than tensor_mul and each per-head swap stalled ~6us) ->
    recip(DVE) -> mul(Pool, its only op class, one resident library).
  - DMA-issue instructions on the scalar queue execute on the ACT
    sequencer and are ring-throttled, so the scalar queue gets only the
    odd x tiles; remaining weights ride sync after the even x tiles or
    the gpsimd SWDGE queue gated behind the last x tile.
  - GroupNorm stats finalize per 4-tile half (gsel is block-diagonal) so
    xn8 and the m0 q/k chains start before the second half's x arrives.

Schedule: flat (head, st) software pipeline with scores leading av by one
step; v^T tiles fill head 0, q/k chains heads 1-3, proj wave A (DR pair
kt0+kt2, needs head 5, shipped as bf16 h) heads 6-7; wave B (DR pair
kt1+kt3, bf16 h3) drains after head 7 with per-half pipelining.  Pairing
the proj waves as (0+2)/(1+3) instead of (0+1)+kt2+kt3 halves their
matmul cycles, removes 8 evacuations from ACT exactly where it was the
serial engine, and drops a whole 2MB output (h2).  Each head epilogue's
pden/recip/asb8 part is deferred one step so its PE ones-matmuls never
head-of-line-block the next head's scores (that stall made HAM
re-throttle the clock at every head boundary).  PSUM: scores
double-buffer (4 banks) + av accumulator/denominator-broadcast slot (2) +
background chain slot (2).
"""

import numpy as np
import ml_dtypes

import concourse.bass as bass
import concourse.tile as tile
from concourse import bacc, mybir
from concourse.bass_utils import run_bass_kernel_spmd

F32 = mybir.dt.float32
BF16 = mybir.dt.bfloat16
F8 = mybir.dt.float8e4
U8 = mybir.dt.uint8
AF = mybir.ActivationFunctionType
ALU = mybir.AluOpType
AX = mybir.AxisListType
DR = mybir.MatmulPerfMode.DoubleRow

B, C, T = 4, 1024, 1024
GROUPS = 32
N_HEADS = 16
CH = C // N_HEADS            # 64
EPS = 1e-5
NCORES = 8
HPC = 8                      # heads per core
CPC = HPC * CH               # 512
CT = C // 128                # 8 c-tiles
TT = T // 128                # 8 t-tiles
GSIZE = C // GROUPS
GN_N = GSIZE * T

SHIFT = 3.0                  # exp(s - SHIFT); cancels in the denominator
WSC = 8.0                    # weight pre-scale before fp8 cast
ASC = 16.0                   # activation (a) pre-scale before fp8 cast
SCALE = float(CH) ** -0.25
L2E = float(np.log2(np.e))
EXP_A = 8.0 * L2E            # DVE bit-trick: bits = round(s*EXP_A + EXP_B)
EXP_B = 8.0 * (7.0 - SHIFT * L2E - 0.0435)

# steps whose exp runs on the DVE (bit trick) instead of ACT, per head.
# Heads 4-7 give the DVE more: that's where waveA/waveBk2 evacs pile onto
# ACT and the pipeline was measurably ACT-serial (balances ACT/DVE at
# ~86us each).
DVE_ST = {h: ((2, 5) if h < 4 else (2, 4, 6)) for h in range(HPC)}

_CACHE = {}


def _build_program():
    nc = bacc.Bacc("TRN2", target_bir_lowering=False, debug=False, num_devices=NCORES)

    names = [
        ("xb", [128, CT, T], BF16),
        ("gsel", [128, 128], BF16),
        ("wq8", [128, CT, CPC], F8),
        ("wk8", [128, CT, CPC], F8),
        ("wv8", [128, CT, CPC], F8),
        ("bqk", [128, 8], F32),
        ("bvb", [128, HPC, CH], BF16),
        ("pt8", [128, 4, C], F8),
    ]
    aps = {}
    for n, shp, dt in names:
        aps[n] = nc.dram_tensor(n, shp, dt, kind="ExternalInput").ap()
    aps["h"] = nc.dram_tensor("h", [CT, 128, T], BF16, kind="ExternalOutput").ap()
    aps["h3"] = nc.dram_tensor("h3", [CT, 128, T], BF16, kind="ExternalOutput").ap()

    with tile.TileContext(nc) as tc:
        _body(tc, aps)
    nc.compile()
    return nc


def _body(tc, aps):
    nc = tc.nc
    with (
        tc.tile_pool(name="wpool", bufs=1) as wpool,
        tc.tile_pool(name="xpool", bufs=1) as xpool,
        tc.tile_pool(name="stats", bufs=1) as stats,
        tc.tile_pool(name="qk", bufs=1) as qk,
        tc.tile_pool(name="probs", bufs=4) as probsp,
        tc.tile_pool(name="bc", bufs=2) as bcp,
        tc.tile_pool(name="hp", bufs=4) as hp,
        tc.tile_pool(name="pp", bufs=1, space="PSUM") as pp,
    ):
        # ---- loads: x tiles alternate the two DMA queues; weights follow
        # (gsel first on scalar -- the stats matmul needs it early) ----
        xb = xpool.tile([128, CT, T], BF16)
        gsel_t = wpool.tile([128, 128], BF16)
        wq8_t = wpool.tile([128, CT, CPC], F8)
        wk8_t = wpool.tile([128, CT, CPC], F8)
        bqk_t = wpool.tile([128, 8], F32)
        wv8_t = wpool.tile([128, CT, CPC], F8)
        bvb_t = wpool.tile([128, HPC, CH], BF16)
        pt8_t = wpool.tile([128, 4, C], F8)
        # DMA queues: gsel first on sync (tiny; the stats matmul gates on
        # it), then x tiles round-robin over FOUR queues (sync, scalar,
        # gpsimd-SWDGE, vector) so the 2MB of x lands in ~1/4 the time;
        # stats half 0 needs tiles 0-3, which land first (one per queue).
        # Weights follow x on each queue, spread so no queue carries more
        # than ~1MB before its last-needed-early tensor.
        # x rides in 2-tile chunks (4KB contiguous per partition -- ~2x the
        # per-queue descriptor throughput of 2KB tiles), one chunk per
        # queue so stats half 0 (tiles 0-3) lands first everywhere.
        # the scalar queue gets ONLY the x23 issue: a second DMA-issue
        # there ring-throttles the ACT sequencer behind x23's descriptors
        # (~4us) and delays the whole stats chain.  wk8/bqk ride sync
        # behind wq8 -- they land ~18us, which is off the critical path
        # (the m0 k-chain is gated by xn8 j3 anyway).
        nc.sync.dma_start(out=gsel_t, in_=aps["gsel"])
        nc.sync.dma_start(out=xb[:, 0:2, :], in_=aps["xb"][:, 0:2, :])
        nc.sync.dma_start(out=wq8_t, in_=aps["wq8"])
        nc.sync.dma_start(out=wk8_t, in_=aps["wk8"])
        nc.sync.dma_start(out=bqk_t, in_=aps["bqk"])
        nc.scalar.dma_start(out=xb[:, 2:4, :], in_=aps["xb"][:, 2:4, :])
        nc.gpsimd.dma_start(out=xb[:, 4:6, :], in_=aps["xb"][:, 4:6, :])
        nc.gpsimd.dma_start(out=xb[:, 6:8, :], in_=aps["xb"][:, 6:8, :])
        nc.gpsimd.dma_start(out=wv8_t, in_=aps["wv8"])
        nc.gpsimd.dma_start(out=bvb_t, in_=aps["bvb"])
        nc.gpsimd.dma_start(out=pt8_t, in_=aps["pt8"])

        # ---- PE warm-up: ~8 junk matmuls gated on a mid-arriving x tile.
        # HAM starts the PE throttled to 1.2GHz and only un-throttles after
        # ~3.4us of sustained busy; these bridge from x-arrival into the
        # stats matmul so the real matmuls run at 2.4GHz.  (Warming too
        # early -- e.g. on gsel at ~8us -- just re-throttles again before
        # the stats burst.)  Rides the bg psum slot; its WAR sem vs the
        # stats matmul is fine. ----
        wps = pp.tile([128, T], F32, tag="bg", name="ps_warm")
        for _ in range(8):
            nc.tensor.matmul(wps[:, 0:512], lhsT=xb[:, 0, 0:128],
                             rhs=xb[:, 0, 0:512], start=True, stop=True)

        epst = wpool.tile([128, 1], F32)
        nc.vector.memset(epst, EPS)
        # dummy activations: pull the ACT table loads (~1.3us each) into
        # the idle DMA window instead of the critical stats/exp path.
        # Square last so its set is resident when the real squares start.
        tldw = wpool.tile([128, 1], F32)
        nc.scalar.activation(out=tldw, in_=epst, func=AF.Exp)
        nc.scalar.activation(out=tldw, in_=epst, func=AF.Identity)
        nc.scalar.activation(out=tldw, in_=epst, func=AF.Square)

        nsh = wpool.tile([128, 1], F32)
        nc.vector.memset(nsh, -SHIFT)
        ascol = wpool.tile([128, 1], F32)
        nc.vector.memset(ascol[0:64], 1.0)
        nc.vector.memset(ascol[64:128], ASC)
        onesr = wpool.tile([1, 128], BF16)
        nc.vector.memset(onesr, 1.0)



        # ---- GroupNorm stats (affine folded into qkv on host).  gsel is
        # block-diagonal (groups live within a c-tile), so stats finalize
        # per 4-tile half: xn8 + the m0 q/k chain segments start ~5us
        # earlier.  sstf layout: [sum0-3 | sq0-3 | sum4-7 | sq4-7]. ----
        # per-tile sum (DVE) + sumsq (ACT) in separate tiles (a shared one
        # serialized the engines through the dependency tracker)
        sstf_s = stats.tile([128, CT], F32)
        sstf_q = stats.tile([128, CT], F32)
        for i in range(CT):
            nc.vector.reduce_sum(out=sstf_s[:, i:i + 1], in_=xb[:, i, :],
                                 axis=AX.X)
            sq = stats.tile([128, T], F32, tag="sq", bufs=2, name="sq")
            nc.scalar.activation(out=sq, in_=xb[:, i, :], func=AF.Square,
                                 accum_out=sstf_q[:, i:i + 1])
        sst = stats.tile([128, 2 * CT], BF16)
        pstat = pp.tile([128, T], F32, tag="bg", name="pstat")
        mean = stats.tile([128, CT], F32)
        var = stats.tile([128, CT], F32)
        std = stats.tile([128, CT], F32)
        rstd = stats.tile([128, CT], F32)
        msq = stats.tile([128, CT], F32)
        nmr = stats.tile([128, CT], F32)

        def emit_stats_half(hf):
            c0 = 4 * hf
            sls = slice(c0, c0 + 4)
            nc.vector.tensor_copy(out=sst[:, 8 * hf:8 * hf + 4],
                                  in_=sstf_s[:, sls])
            nc.vector.tensor_copy(out=sst[:, 8 * hf + 4:8 * hf + 8],
                                  in_=sstf_q[:, sls])
            nc.tensor.matmul(pstat[:, 8 * hf:8 * hf + 8], lhsT=gsel_t,
                             rhs=sst[:, 8 * hf:8 * hf + 8],
                             start=True, stop=True)
            nc.vector.tensor_scalar_mul(mean[:, sls],
                                        pstat[:, 8 * hf:8 * hf + 4],
                                        1.0 / GN_N)
            nc.vector.tensor_mul(msq[:, sls], mean[:, sls], mean[:, sls])
            nc.vector.scalar_tensor_tensor(out=var[:, sls],
                                           in0=pstat[:, 8 * hf + 4:8 * hf + 8],
                                           scalar=1.0 / GN_N, in1=msq[:, sls],
                                           op0=ALU.mult, op1=ALU.subtract)
            nc.scalar.activation(out=std[:, sls], in_=var[:, sls],
                                 func=AF.Sqrt, bias=epst)
            nc.vector.reciprocal_approx_fast(out=rstd[:, sls],
                                             in_=std[:, sls])
            nc.vector.scalar_tensor_tensor(out=nmr[:, sls], in0=mean[:, sls],
                                           scalar=-1.0, in1=rstd[:, sls],
                                           op0=ALU.mult, op1=ALU.mult)

        # ---- persistent activation tiles ----
        xn8 = xpool.tile([128, 4, 2, 2, 512], F8)
        qsb = qk.tile([128, 4, T], BF16)
        ksb = qk.tile([128, 4, T], BF16)
        asb8 = qk.tile([128, 2, 2, 2, 512], F8)
        # v^T, DR-packed: [s(128), st-pair, plane, head, col].  Col 0 = ones:
        # the softmax denominator lands on psum partition 0, where gpsimd
        # partition_broadcast reads.  Cols 1-63 zero pad, 64-127 = v: the num
        # rows sit at base partition 64 (a 64-partition Pool access must be
        # 64-aligned).
        # memsets ride Pool (memset ucode is resident from init; DVE was
        # paying 3.4us for the zero-fill it can't afford mid-front)
        vt8 = qk.tile([128, 4, 2, HPC, 128], F8)
        nc.gpsimd.memset(vt8[:, :, :, :, 0:1], 1.0)
        nc.gpsimd.memset(vt8[:, :, :, :, 1:64], 0.0)

        def psum_tile(tag, bufs=1):
            return pp.tile([128, T], F32, tag=tag, bufs=bufs, name=f"ps_{tag}")

        # ---- xn8 (fp8) + m0 q/k chains, pipelined per stats-half ----
        qm0 = psum_tile("sc", 2)
        km0 = psum_tile("sc", 2)
        for j in range(4):
            if j % 2 == 0:
                emit_stats_half(j // 2)
            i0, i1 = 2 * j, 2 * j + 1
            # xn8 split 4 DVE / 4 ACT: balances the two front EW chains
            # (DVE: 8 reduces + 4 xn8 ~= ACT: 8 squares + 4 xn8)
            if j < 2:
                nc.vector.tensor_scalar(out=xn8[:, j, :, 0, :],
                                        in0=xb[:, i0, :],
                                        scalar1=mean[:, i0:i0 + 1],
                                        scalar2=rstd[:, i0:i0 + 1],
                                        op0=ALU.subtract, op1=ALU.mult)
                nc.vector.tensor_scalar(out=xn8[:, j, :, 1, :],
                                        in0=xb[:, i1, :],
                                        scalar1=mean[:, i1:i1 + 1],
                                        scalar2=rstd[:, i1:i1 + 1],
                                        op0=ALU.subtract, op1=ALU.mult)
            else:
                nc.scalar.activation(out=xn8[:, j, :, 0, :], in_=xb[:, i0, :],
                                     func=AF.Identity, bias=nmr[:, i0:i0 + 1],
                                     scale=rstd[:, i0:i0 + 1])
                nc.scalar.activation(out=xn8[:, j, :, 1, :], in_=xb[:, i1, :],
                                     func=AF.Identity, bias=nmr[:, i1:i1 + 1],
                                     scale=rstd[:, i1:i1 + 1])
            for ps, wt in ((qm0, wq8_t), (km0, wk8_t)):
                for n2 in range(2):
                    nc.tensor.matmul(
                        ps[:, n2 * 512:(n2 + 1) * 512],
                        lhsT=wt[:, i0:i0 + 2, 0:128],
                        rhs=xn8[:, j, n2, :, :],
                        perf_mode=DR, start=(j == 0), stop=(j == 3))
        nc.vector.tensor_scalar(out=qsb[:, 0, :], in0=qm0,
                                scalar1=SCALE / WSC, scalar2=bqk_t[:, 0:1],
                                op0=ALU.mult, op1=ALU.add)
        nc.scalar.activation(out=ksb[:, 0, :], in_=km0, func=AF.Identity,
                             bias=bqk_t[:, 4:5], scale=SCALE / WSC)

        def emit_vt(tt):
            ps = psum_tile("bg")
            n2, tb = tt // 4, tt % 4
            for j in range(4):
                nc.tensor.matmul(
                    ps[:, 0:CPC],
                    lhsT=xn8[:, j, n2, :, tb * 128:(tb + 1) * 128],
                    rhs=wv8_t[:, 2 * j:2 * j + 2, :],
                    perf_mode=DR, start=(j == 0), stop=(j == 3))
            nc.vector.scalar_tensor_tensor(
                out=vt8[:, tt // 2, tt % 2, :, 64:128],
                in0=ps[:, 0:CPC].rearrange("p (h c) -> p h c", h=HPC),
                scalar=1.0 / WSC, in1=bvb_t, op0=ALU.mult, op1=ALU.add)

        qk_chain = {}

        def emit_qk_seg(m, which, j):
            wt, bcol, dst = ((wq8_t, m, qsb), (wk8_t, 4 + m, ksb))[which]
            key = (m, which)
            if key not in qk_chain:
                qk_chain[key] = psum_tile("bg")
            ps = qk_chain[key]
            for n2 in range(2):
                nc.tensor.matmul(
                    ps[:, n2 * 512:(n2 + 1) * 512],
                    lhsT=wt[:, 2 * j:2 * j + 2, m * 128:(m + 1) * 128],
                    rhs=xn8[:, j, n2, :, :],
                    perf_mode=DR, start=(j == 0), stop=(j == 3))
            if j == 3:
                nc.vector.tensor_scalar(out=dst[:, m, :], in0=ps,
                                        scalar1=SCALE / WSC,
                                        scalar2=bqk_t[:, bcol:bcol + 1],
                                        op0=ALU.mult, op1=ALU.add)
                del qk_chain[key]

        def emit_waveA(ot):
            # DR pair (kt0, kt2) = heads 0,1,4,5: runs during heads 6-7
            # (needs head 5's epilogue).  Residual + proj bias are added
            # on the HOST (it sums the partials anyway), so h ships bf16.
            ps = psum_tile("bg")
            for n2 in range(2):
                sl = slice(n2 * 512, (n2 + 1) * 512)
                nc.tensor.matmul(
                    ps[:, sl],
                    lhsT=pt8_t[:, 0:2, ot * 128:(ot + 1) * 128],
                    rhs=asb8[:, 0, n2, :, :],
                    perf_mode=DR, start=True, stop=True)
            ho = hp.tile([128, T], BF16, tag="ho")
            nc.scalar.activation(out=ho, in_=ps, func=AF.Identity,
                                 scale=1.0 / (WSC * ASC))
            nc.sync.dma_start(out=aps["h"][ot], in_=ho)

        wb_state = {}

        def emit_waveB_half(ot, n2):
            srcs = [("sc", 2), ("sc", 2), ("bg", 1), ("pa", 1)]
            if n2 == 0:
                tag, bufs = srcs[ot % 4]
                wb_state[ot] = (psum_tile(tag, bufs),
                                hp.tile([128, T], BF16, tag="hs", name="hs"))
            ps, hs = wb_state[ot]
            sl = slice(n2 * 512, (n2 + 1) * 512)
            nc.tensor.matmul(ps[:, sl],
                             lhsT=pt8_t[:, 2:4, ot * 128:(ot + 1) * 128],
                             rhs=asb8[:, 1, n2, :, :],
                             perf_mode=DR, start=True, stop=True)
            if n2 == 0:
                nc.vector.tensor_scalar_mul(hs[:, sl], ps[:, sl],
                                            1.0 / (WSC * ASC))
            else:
                nc.scalar.activation(out=hs[:, sl], in_=ps[:, sl],
                                     func=AF.Identity, scale=1.0 / (WSC * ASC))
                del wb_state[ot]
            # ship each half as soon as it exists: n2=0 halves on sync;
            # n2=1 halves split gpsimd/scalar (all n2=1 emits come after
            # the last asb8 mul in Pool program order, so no head-of-line
            # risk; ACT's tail is light enough to afford 4 issues)
            if n2 == 0:
                eng = nc.sync
            else:
                eng = nc.gpsimd if ot % 2 == 0 else nc.scalar
            eng.dma_start(out=aps["h3"][ot][:, sl], in_=hs[:, sl])

        # Background schedule: (head, st) -> closures run between that
        # step's scores and the lagged av, where PE would otherwise wait.
        sched = {}

        def at(h, st, fn):
            sched.setdefault((h, st), []).append(fn)

        for j in range(6):
            at(0, j, lambda tt=j + 2: emit_vt(tt))
        for m in (1, 2, 3):
            for j in range(4):
                at(m, j, lambda m=m, j=j: emit_qk_seg(m, 0, j))
                at(m, 4 + j, lambda m=m, j=j: emit_qk_seg(m, 1, j))
        # wave A' (kt0+kt2 DR) needs head 5's epilogue -> heads 6-7 slots
        for ot in range(CT):
            at(6 + ot // 4, 1 + 2 * (ot % 4), lambda ot=ot: emit_waveA(ot))

        # ---- lead-in v^T tiles ----
        for tt in range(2):
            emit_vt(tt)

        # ---- attention: flat (head, st) pipeline, scores lead av by 1 ----
        steps = [(h, st) for h in range(HPC) for st in range(TT)]
        pa_tiles = {}
        pr_tiles = {}

        def emit_scores(k):
            h, st = steps[k]
            m, po = h // 2, CH * (h % 2)
            if st % 2 == 0:
                pr_tiles[k // 2] = probsp.tile([128, 2, 2, 512], F8, tag="pr",
                                               name="prt")
            ps = psum_tile("sc", 2)
            for n2 in range(2):
                nc.tensor.matmul(
                    ps[:, n2 * 512:(n2 + 1) * 512],
                    lhsT=ksb[po:po + CH, m, st * 128:(st + 1) * 128],
                    rhs=qsb[po:po + CH, m, n2 * 512:(n2 + 1) * 512],
                    start=True, stop=True)
            pr = pr_tiles[k // 2]
            psr = ps.rearrange("p (a n) -> p a n", a=2)
            if st in DVE_ST[h]:
                nc.vector.tensor_scalar(out=pr[:, :, st % 2, :].bitcast(U8),
                                        in0=psr, scalar1=EXP_A, scalar2=EXP_B,
                                        op0=ALU.mult, op1=ALU.add)
            else:
                nc.scalar.activation(out=pr[:, :, st % 2, :], in_=psr,
                                     func=AF.Exp, bias=nsh)

        def emit_av(k):
            h, st = steps[k]
            m, po = h // 2, CH * (h % 2)
            if st % 2 == 0:
                return
            j = st // 2
            pr = pr_tiles.pop(k // 2)
            if j == 0:
                # heads 4-5 accumulate in the bg slot (free there: no
                # chains or waves), so their pa allocation never WARs the
                # previous head's pden/rc2 chain -- that WAR stalled the
                # av stream ~4us at the boundaries into heads 4-6
                pa_tiles[h] = pp.tile([128, T], F32, tag="pa", bufs=1,
                                      name="pat")
            pa = pa_tiles[h]
            for n2 in range(2):
                nc.tensor.matmul(
                    pa[0:128, n2 * 512:(n2 + 1) * 512],
                    lhsT=vt8[:, j, :, h, :],
                    rhs=pr[:, n2, :, :],
                    perf_mode=DR, start=(j == 0), stop=(j == 3))
            if st != TT - 1:
                return
            # head epilogue part 1: pa row 0 = denominator, rows 64-127 =
            # num*ASC after the ascol-scaled evac (frees the single pa
            # slot).  Part 2 (pden broadcast / recip / asb8) is DEFERRED
            # one step: its PE ones-matmuls wait on this evac, and emitted
            # inline they head-of-line-blocked the next head's scores for
            # >1us at EVERY head boundary -- exactly the per-head PE idle
            # that made HAM re-throttle the clock to 1.2GHz mid-kernel.
            af = bcp.tile([128, T], BF16, tag="af")
            rc2 = bcp.tile([128, T], F32, tag="rc2")
            nc.vector.tensor_scalar_mul(af[:, 0:512], pa[0:128, 0:512], ascol)
            if h < HPC - 1:
                nc.vector.tensor_scalar_mul(af[:, 512:T], pa[0:128, 512:T],
                                            ascol)
            else:
                nc.scalar.activation(out=af[:, 512:T], in_=pa[0:128, 512:T],
                                     func=AF.Identity, scale=ascol)
            epi_pend.append((h, af, rc2))

        epi_pend = []

        def epi_part2(h, af, rc2):
            m, po = h // 2, CH * (h % 2)
            # pden reuses the tag its head's pa lived in (just freed by
            # the af evac), never the next head's slot
            pden = pp.tile([128, T], F32, tag="pa", bufs=1, name="pden")
            for nb in range(2):
                sl = slice(nb * 512, (nb + 1) * 512)
                nc.tensor.matmul(pden[:, sl], lhsT=onesr, rhs=af[0:1, sl],
                                 start=True, stop=True)
                nc.vector.reciprocal_approx_fast(out=rc2[:, sl],
                                                 in_=pden[:, sl])
                # asb8 index: [po, wave m%2, nb, plane m//2, :] -- each
                # proj wave is a DR pair (kt0+kt2) / (kt1+kt3) and DR
                # wants its two planes adjacent (strided pairs are 2x
                # slower), so heads interleave by m parity
                nc.gpsimd.tensor_mul(
                    out=asb8[po:po + CH, m % 2, nb, m // 2, :],
                    in0=af[64:128, sl], in1=rc2[64:128, sl])
                if h == HPC - 1 and nb == 0:
                    # kick wave B's first halves as soon as its n2=0
                    # planes exist
                    for ot in range(2):
                        emit_waveB_half(ot, 0)

        LEAD = 1
        for k in range(64 + LEAD):
            if k < 64:
                emit_scores(k)
            if epi_pend:
                # before the sched closures: waveA/waveBk2 in this slot
                # read the asb8 this writes
                epi_part2(*epi_pend.pop(0))
            if k < 64:
                for fn in sched.get(steps[k], ()):
                    fn()
            if k >= LEAD:
                emit_av(k - LEAD)
        while epi_pend:
            epi_part2(*epi_pend.pop(0))

        # ---- wave B tail: DR pair kt1+kt3 (kt0+kt2 shipped during heads
        # 6-7); psum slots rotate so ot chains overlap; evacs DVE/ACT ----
        for ot in range(2, CT):
            emit_waveB_half(ot, 0)
        for ot in range(CT):
            emit_waveB_half(ot, 1)


def _pack_inputs(x, gn_weight, gn_bias, qkv_w, qkv_b, proj_w, proj_b):
    """Build the 8 per-core input dicts (host-side packing only)."""
    bf = ml_dtypes.bfloat16
    f8 = ml_dtypes.float8_e4m3
    gsel = np.kron(np.eye(4, dtype=np.float32),
                   np.ones((GSIZE, GSIZE), dtype=np.float32)).astype(bf)
    # Fold GroupNorm affine into the qkv conv (exact):
    qkv_b = (qkv_b.astype(np.float64) +
             qkv_w.astype(np.float64) @ gn_bias.astype(np.float64)
             ).astype(np.float32)
    qkv_w = (qkv_w * gn_weight[None, :]).astype(np.float32)

    in_maps = []
    for core in range(NCORES):
        b_idx, g = core // 2, core % 2
        hh = np.arange(CPC) // CH + HPC * g
        cc = np.arange(CPC) % CH
        qrows = 192 * hh + cc
        krows = qrows + CH
        vrows = qrows + 2 * CH

        def packT(rows):
            w = (qkv_w[rows, :] * WSC).T.astype(f8)          # [C, CPC]
            return np.ascontiguousarray(
                w.reshape(CT, 128, CPC).transpose(1, 0, 2))  # [128, CT, CPC]

        bqv = np.concatenate(
            [(qkv_b[qrows] * SCALE).reshape(4, 128).T,
             (qkv_b[krows] * SCALE).reshape(4, 128).T], axis=1)
        bvv = np.ascontiguousarray(np.broadcast_to(
            qkv_b[vrows].reshape(1, HPC, CH), (128, HPC, CH))).astype(bf)

        ptm = (proj_w[:, g * CPC:(g + 1) * CPC].T * WSC).astype(f8)
        # kt plane order (0, 2, 1, 3): the two proj waves are DR pairs
        # (kt0+kt2) and (kt1+kt3)
        ptm = np.ascontiguousarray(
            ptm.reshape(4, 128, C)[[0, 2, 1, 3]].transpose(1, 0, 2))

        xin = np.ascontiguousarray(
            x[b_idx].reshape(CT, 128, T).transpose(1, 0, 2).astype(bf))

        in_maps.append({
            "xb": xin,
            "gsel": gsel,
            "wq8": packT(qrows),
            "wk8": packT(krows),
            "wv8": packT(vrows),
            "bqk": np.ascontiguousarray(bqv.astype(np.float32)),
            "bvb": bvv,
            "pt8": ptm,
        })
    return in_maps


def kernel(x, gn_weight, gn_bias, qkv_w, qkv_b, proj_w, proj_b, **run_kwargs):
    x = np.asarray(x, dtype=np.float32)
    gn_weight = np.asarray(gn_weight, dtype=np.float32)
    gn_bias = np.asarray(gn_bias, dtype=np.float32)
    qkv_w = np.asarray(qkv_w, dtype=np.float32)
    qkv_b = np.asarray(qkv_b, dtype=np.float32)
    proj_w = np.asarray(proj_w, dtype=np.float32)
    proj_b = np.asarray(proj_b, dtype=np.float32)

    if "nc" not in _CACHE:
        _CACHE["nc"] = _build_program()
    nc = _CACHE["nc"]

    in_maps = _pack_inputs(x, gn_weight, gn_bias, qkv_w, qkv_b, proj_w, proj_b)
    res = run_bass_kernel_spmd(nc, in_maps, core_ids=list(range(NCORES)),
                               **run_kwargs)
    out = np.empty((B, C, T), dtype=np.float32)
    for b_idx in range(B):
        r0, r1 = res.results[2 * b_idx], res.results[2 * b_idx + 1]
        acc = x[b_idx] + proj_b[:, None]
        for r in (r0, r1):
            for nm in ("h", "h3"):
                acc = acc + np.asarray(r[nm]).reshape(C, T).astype(np.float32)
        out[b_idx] = acc
    if run_kwargs:
        return out, res
    return out

